# revision 5
# baseline (speedup 1.0000x reference)
"""Distributed 6-layer GCN (gcn_norm with self-loops) for 8 TRN2 NeuronCores, v2.

Layout: 8 ranks x 12800 slots (100 tiles of 128). Tiles 0-49 = half 0,
50-99 = half 1; partition parity p//64 picks the 64-bf16 half of a 256B
table pair-row. Source nodes are 4-bin colored (half x parity) by a
vectorized local search so each destination's in-edges split evenly;
tiles pack nodes with similar in-count vectors so per-(tile,bin) column
counts (shared across ranks) pad minimally.

Per layer:
    z    = x @ W              (PE bf16; layer 5 aggregates first)
    hs   = dinv * z           (bf16, node-major)
    2x AllGather (Shared) of pair-rows -> tab_h [25600, 128] bf16
    acc  = sum of dma_gather'd 256B rows via PE identity-MM accumulate
    out  = dinv * (acc + hs) + b ; leaky_relu(0.2) between layers
"""
import os

import numpy as np

ABLATE = os.environ.get("ABLATE", "")

N = 100000
E = 1600000
R = 8
DIN, DH, DOUT = 128, 64, 4
NHALF = 2
NBIN = 4                      # (half, parity)
TPH = 50                      # tiles per half
TILES = NHALF * TPH           # 100
SHARD = TILES * 128           # 12800
PAIRS = TPH * 64              # 3200 pair-rows per (rank, half)
WINROWS = R * PAIRS           # 25600 rows per window table
ZPAIR = PAIRS - 1             # reserved zero pair (rank 0) -> idx 3199
GROUP_CAP = 504               # nodes per (tile, parity) group (pr 63 unused)
BIN_CAP = TPH * GROUP_CAP     # 25200 nodes per (half, parity) bin
CHUNKCOLS = 62
NQUEUES = 4

_cache = {}
_last_maps = None


# ----------------------------------------------------------------- host prep
def _assign_bins(src, dst, rng):
    """4-bin source coloring minimizing sum_d sum_b c_b(d)^2 (c = per-dst
    in-edge count by source bin), capacity BIN_CAP per bin."""
    cap = BIN_CAP
    bins = rng.integers(0, NBIN, N).astype(np.int32)
    outdeg = np.bincount(src, minlength=N).astype(np.int64)

    def counts(b):
        return (
            np.bincount(b[src].astype(np.int64) * N + dst, minlength=NBIN * N)
            .reshape(NBIN, N)
            .astype(np.int64)
        )

    c = counts(bins)
    best = ((c * c).sum(), bins.copy())
    idx = np.arange(N)
    for frac in (0.5, 0.5, 0.4, 0.4, 0.3, 0.3, 0.25, 0.2, 0.2, 0.15,
                 0.12, 0.1, 0.1, 0.08, 0.08, 0.06):
        S = np.empty((NBIN, N))
        for b in range(NBIN):
            S[b] = np.bincount(src, weights=c[b, dst].astype(np.float64),
                               minlength=N)
        gain = S - S[bins, idx][None, :] + outdeg[None, :]
        gain[bins, idx] = 0.0
        bb = gain.argmin(axis=0).astype(np.int32)
        gb = gain[bb, idx]
        move = (gb < 0) & (rng.random(N) < frac)
        bins = bins.copy()
        bins[move] = bb[move]
        c = counts(bins)
        obj = (c * c).sum()
        if obj < best[0]:
            best = (obj, bins.copy())
    bins = best[1]
    c = counts(bins)

    # sequential chunked best-response sweeps (exact incremental updates)
    o = np.argsort(src, kind="stable")
    odst = dst[o].astype(np.int64)
    optr = np.zeros(N + 1, np.int64)
    np.cumsum(np.bincount(src[o], minlength=N), out=optr[1:])
    CH = 2000
    for sweep in range(6):
        perm = rng.permutation(N)
        for i0 in range(0, N, CH):
            nodes = perm[i0:i0 + CH]
            lens = (optr[nodes + 1] - optr[nodes]).astype(np.int64)
            if lens.sum() == 0:
                continue
            tg = np.concatenate([odst[optr[s]:optr[s + 1]] for s in nodes])
            owner = np.repeat(np.arange(len(nodes)), lens)
            S = np.zeros((NBIN, len(nodes)))
            for b in range(NBIN):
                S[b] = np.bincount(owner, weights=c[b, tg].astype(np.float64),
                                   minlength=len(nodes))
            cur = bins[nodes]
            li = np.arange(len(nodes))
            gain = S - S[cur, li][None, :] + lens[None, :]
            gain[cur, li] = 0.0
            bb = gain.argmin(axis=0).astype(np.int32)
            gb = gain[bb, li]
            mv = gb < 0
            if not mv.any():
                continue
            mvn = nodes[mv]
            oldb = bins[mvn].astype(np.int64)
            newb = bb[mv].astype(np.int64)
            bins[mvn] = bb[mv]
            mlens = (optr[mvn + 1] - optr[mvn]).astype(np.int64)
            mtg = np.concatenate([odst[optr[s]:optr[s + 1]] for s in mvn])
            np.add.at(c.ravel(), np.repeat(oldb, mlens) * N + mtg, -1)
            np.add.at(c.ravel(), np.repeat(newb, mlens) * N + mtg, 1)
    # capacity fix: move cheapest nodes from over-full to under-full bins
    sizes = np.bincount(bins, minlength=NBIN)
    while (sizes > cap).any():
        b_over = int(sizes.argmax())
        b_to = int(sizes.argmin())
        S_over = np.bincount(src, weights=c[b_over, dst].astype(np.float64),
                             minlength=N)
        S_to = np.bincount(src, weights=c[b_to, dst].astype(np.float64),
                           minlength=N)
        pen = np.where(bins == b_over, S_to - S_over + outdeg, np.inf)
        n_move = int(sizes[b_over] - cap)
        mv = np.argpartition(pen, n_move)[:n_move]
        bins[mv] = b_to
        c = counts(bins)
        sizes = np.bincount(bins, minlength=NBIN)
    return bins, c


def _pack_groups(v, ngroups, cap):
    """Greedy vector bin packing of columns of v [4, n] into ngroups groups
    of <= cap, minimizing sum of per-group componentwise maxima. Batched by
    distinct pattern. Returns (group_of [n], Kg [ngroups, 4])."""
    n = v.shape[1]
    pats, inv, cnts = np.unique(v.T, axis=0, return_inverse=True,
                                return_counts=True)
    porder = np.lexsort((-pats.sum(axis=1), -pats.max(axis=1)))
    Kg = np.zeros((ngroups, 4), np.int64)
    fill = np.zeros(ngroups, np.int64)
    runs = {int(p): [] for p in range(len(pats))}
    for p in porder:
        rem = int(cnts[p])
        vi = pats[p]
        while rem > 0:
            inc = np.maximum(Kg, vi[None, :]).sum(axis=1) - Kg.sum(axis=1)
            inc = np.where(fill >= cap, 1 << 30, inc)
            g = int(np.lexsort((-fill, inc))[0])
            take = int(min(rem, cap - fill[g]))
            runs[int(p)].append((g, take))
            Kg[g] = np.maximum(Kg[g], vi)
            fill[g] += take
            rem -= take
    group_of = np.empty(n, np.int64)
    for p in range(len(pats)):
        members = np.where(inv == p)[0]
        off = 0
        for (g, take) in runs[p]:
            group_of[members[off:off + take]] = g
            off += take
    return group_of, Kg


def _build_graph(edge_index):
    rng = np.random.default_rng(12345)
    src = edge_index[0].astype(np.int64)
    dst = edge_index[1].astype(np.int64)
    indeg = np.bincount(dst, minlength=N)
    dinv = (1.0 / np.sqrt(indeg + 1.0)).astype(np.float32)

    bins, c = _assign_bins(src, dst, rng)

    # --- phase B: greedy vector packing into (tile, parity) groups of
    # <= GROUP_CAP (pr 63 never used -> pair k=63 mod 64 rows stay zero);
    # pair parity groups into tiles by sorted group weight.
    slot_of_node = np.full(N, -1, np.int64)   # rank*SHARD + pos
    for h in range(NHALF):
        ga = {}
        Kga = {}
        for par in range(2):
            nodes = np.where(bins == 2 * h + par)[0]
            g_of, Kg = _pack_groups(c[:, nodes], TPH, GROUP_CAP)
            ga[par] = (nodes, g_of)
            Kga[par] = Kg
        # pair heaviest-with-heaviest: tile-local i <- parity group o[par][i]
        o0 = np.argsort(-Kga[0].sum(axis=1), kind="stable")
        o1 = np.argsort(-Kga[1].sum(axis=1), kind="stable")
        tl_of_group = {0: np.empty(TPH, np.int64), 1: np.empty(TPH, np.int64)}
        tl_of_group[0][o0] = np.arange(TPH)
        tl_of_group[1][o1] = np.arange(TPH)
        for par in range(2):
            nodes, g_of = ga[par]
            order = np.argsort(g_of, kind="stable")
            nodes = nodes[order]
            g_sorted = g_of[order]
            # within-group sequence number
            seq = np.arange(len(nodes)) - np.searchsorted(
                g_sorted, g_sorted)
            rank = seq % 8
            pr = seq // 8
            assert pr.max() < 63
            tile = h * TPH + tl_of_group[par][g_sorted]
            part = par * 64 + pr
            slot_of_node[nodes] = rank * SHARD + tile * 128 + part
    assert (slot_of_node >= 0).all()

    dinv_slot = np.zeros(R * SHARD, np.float32)
    dinv_slot[slot_of_node] = dinv

    # --- source pair-row index (within window h_s): rank*PAIRS + local pair
    sslot = slot_of_node[src]
    s_rank = sslot // SHARD
    s_pos = sslot % SHARD
    s_tile = s_pos // 128
    s_part = s_pos % 128
    s_half = s_tile // TPH
    s_par = s_part // 64
    s_row = s_rank * PAIRS + (s_tile - s_half * TPH) * 64 + (s_part % 64)
    s_bin = s_half * 2 + s_par

    dslot = slot_of_node[dst]
    d_rank = dslot // SHARD
    d_pos = dslot % SHARD
    d_tile = d_pos // 128
    d_part = d_pos % 128

    # per (rank, tile, bin, partition) counts -> K[t, b] global max
    key = ((d_rank * TILES + d_tile) * NBIN + s_bin) * 128 + d_part
    eorder = np.argsort(key, kind="stable")
    key_s = key[eorder]
    srow_s = s_row[eorder]
    cnt = np.bincount(key_s, minlength=R * TILES * NBIN * 128).reshape(
        R, TILES, NBIN, 128)
    K = cnt.max(axis=3).max(axis=0)          # [TILES, NBIN]

    # --- column layout: stripes (dq: 25 tiles) x window h'; cols per
    # (t, b) contiguous, order (h', t, par'); chunks of <=CHUNKCOLS cols.
    colbase = np.zeros((TILES, NBIN), np.int64)
    stripes = []
    total_cols = 0
    for dq in range(4):
        tl = list(range(25 * dq, 25 * dq + 25))
        wins = []
        for h in range(NHALF):
            c0w = total_cols
            segs_of_tile = {}
            for t in tl:
                segs_of_tile[t] = []
                for par in range(2):
                    b = 2 * h + par
                    colbase[t, b] = total_cols
                    if K[t, b] > 0:
                        segs_of_tile[t].append(
                            (total_cols, int(K[t, b]), par))
                    total_cols += int(K[t, b])
            ncw = total_cols - c0w
            chunks = []
            off = 0
            while off < ncw:
                n = min(CHUNKCOLS, ncw - off)
                chunks.append((c0w + off, n))
                off += n
            wins.append(dict(col0=c0w, ncols=ncw, chunks=chunks,
                             segs=segs_of_tile))
        stripes.append(dict(tiles=tl, wins=wins))
    NIDX = total_cols * 128

    # --- fill idx
    idx = np.full((R, total_cols, 128), ZPAIR, np.int64)
    ptr = np.zeros(R * TILES * NBIN * 128 + 1, np.int64)
    np.cumsum(cnt.ravel(), out=ptr[1:])
    rib = np.arange(E) - ptr[key_s]
    er = key_s // (TILES * NBIN * 128)
    erem = key_s % (TILES * NBIN * 128)
    et = erem // (NBIN * 128)
    eb = (erem // 128) % NBIN
    ep = erem % 128
    ecol = colbase[et, eb] + rib
    idx[er, ecol, ep] = srow_s

    idx_flat = idx.reshape(R, NIDX)
    wrapped = idx_flat.reshape(R, NIDX // 16, 16).transpose(0, 2, 1)
    idx_tiles = np.tile(wrapped, (1, 8, 1)).astype(np.int16)

    pos_grid = np.arange(SHARD).reshape(TILES, 128)
    dinv_rt = np.zeros((R, 128, TILES), np.float32)
    for r in range(R):
        dinv_rt[r] = dinv_slot[r * SHARD + pos_grid].T

    return dict(
        slot_of_node=slot_of_node, idx_tiles=idx_tiles, dinv_rt=dinv_rt,
        stripes=stripes, NIDX=NIDX, total_cols=total_cols,
        padding=NIDX * R / float(E),
    )


# ------------------------------------------------------------- bass program
def _build_program(meta, with_bias):
    from concourse import bacc, bass, mybir, tile
    from concourse.masks import make_identity

    NIDX = meta["NIDX"]
    BF = mybir.dt.bfloat16
    F32 = mybir.dt.float32
    nc = bacc.Bacc("TRN2", target_bir_lowering=False, debug=False,
                   num_devices=R, num_swdge_queues=NQUEUES)

    xs_d = nc.dram_tensor("xs", [DIN, SHARD], BF, kind="ExternalInput")
    gidx_d = nc.dram_tensor("gidx", [128, NIDX // 16], mybir.dt.int16,
                            kind="ExternalInput")
    dinv_d = nc.dram_tensor("dinv", [128, TILES], F32, kind="ExternalInput")
    W_d = [nc.dram_tensor(f"W{i}", [DIN if i == 0 else DH,
                                    DOUT if i == 5 else DH],
                          F32, kind="ExternalInput") for i in range(6)]
    if with_bias:
        bb_d = nc.dram_tensor("bb", [5 * 128, DH], F32, kind="ExternalInput")
    out_d = nc.dram_tensor("out", [SHARD, DOUT], F32, kind="ExternalOutput")

    ag_d = [nc.dram_tensor(f"ag{h}", [PAIRS, 2 * DH], BF)
            for h in range(NHALF)]
    tab_d = [nc.dram_tensor(f"tab{h}", [WINROWS, 2 * DH], BF,
                            addr_space="Shared") for h in range(NHALF)]

    AL = mybir.AluOpType
    with tile.TileContext(nc) as tc:
        with (
            tc.tile_pool(name="const", bufs=1) as constp,
            tc.tile_pool(name="persist", bufs=1) as persist,
            tc.tile_pool(name="x0p", bufs=3) as x0p,
            tc.tile_pool(name="xtp", bufs=3) as xtp,
            tc.tile_pool(name="gbuf", bufs=4) as gbuf,
            tc.tile_pool(name="ep", bufs=6) as epp,
            tc.tile_pool(name="ps_acc", bufs=4, space="PSUM") as ps_acc,
            tc.tile_pool(name="ps_tr", bufs=2, space="PSUM") as ps_tr,
            tc.tile_pool(name="ps_h", bufs=2, space="PSUM") as ps_h,
        ):
            identb = constp.tile([128, 128], BF)
            make_identity(nc, identb[:])
            idx_t = persist.tile([128, NIDX // 16], mybir.dt.int16)
            nc.sync.dma_start(out=idx_t[:], in_=gidx_d[:])
            dinv_t = constp.tile([128, TILES], F32)
            nc.sync.dma_start(out=dinv_t[:], in_=dinv_d[:])
            dinv02_t = constp.tile([128, TILES], F32)
            nc.scalar.mul(out=dinv02_t[:], in_=dinv_t[:], mul=0.2)
            W_t = []
            for i in range(6):
                wt = constp.tile(list(W_d[i].shape), BF, tag=f"W{i}")
                nc.gpsimd.dma_start(out=wt[:], in_=W_d[i][:])
                W_t.append(wt)
            if with_bias:
                bb_t = constp.tile([128, 5 * DH], F32)
                nc.sync.dma_start(
                    out=bb_t[:].rearrange("p (l d) -> p l d", d=DH),
                    in_=bb_d[:].rearrange("(l p) d -> p l d", p=128))

            hs_buf = persist.tile([128, TILES * DH], BF)
            accs = persist.tile([128, TILES * DH], BF)
            xa = persist.tile([128, TILES * DH], BF)
            xb = persist.tile([128, TILES * DH], BF)
            out_sb = persist.tile([128, TILES * DOUT], F32)

            def dinv_col(t):
                return dinv_t[:, t:t + 1].to_broadcast([128, DH])

            qn = [0]

            def next_q():
                q = qn[0] % NQUEUES
                qn[0] += 1
                return q

            def epilogue(l, t, acc, xdst):
                """Final per-tile combine after window-1 accumulation.
                acc = window-1 PSUM partial (or None); accs[t] already
                holds window-0 partial + hs (self-loop term)."""
                t2 = epp.tile([128, DH], F32, tag="t2")
                if acc is None:
                    nc.vector.tensor_copy(
                        out=t2[:], in_=accs[:, t * DH:(t + 1) * DH])
                else:
                    nc.vector.tensor_tensor(
                        out=t2[:], in0=acc[:],
                        in1=accs[:, t * DH:(t + 1) * DH], op=AL.add)
                if l < 5:
                    a2 = epp.tile([128, DH], F32, tag="a2")
                    nc.vector.tensor_tensor(
                        out=a2[:], in0=t2[:], in1=dinv_col(t), op=AL.mult)
                    t3 = epp.tile([128, DH], F32, tag="t3")
                    if with_bias:
                        nc.vector.tensor_tensor(
                            out=a2[:], in0=a2[:],
                            in1=bb_t[:, l * DH:(l + 1) * DH], op=AL.add)
                        nc.scalar.mul(out=t3[:], in_=a2[:], mul=0.2)
                    else:
                        nc.scalar.activation(
                            out=t3[:], in_=t2[:],
                            func=mybir.ActivationFunctionType.Copy,
                            scale=dinv02_t[:, t:t + 1])
                    nc.vector.tensor_tensor(
                        out=xdst[:, t * DH:(t + 1) * DH],
                        in0=a2[:], in1=t3[:], op=AL.max)
                else:
                    agg = epp.tile([128, DH], BF, tag="a2")
                    nc.vector.tensor_tensor(
                        out=agg[:], in0=t2[:], in1=dinv_col(t), op=AL.mult)
                    tr = ps_tr.tile([128, 128], BF, space="PSUM", tag="tr")
                    nc.tensor.transpose(out=tr[:DH, :], in_=agg[:],
                                        identity=identb[:])
                    aggT = xtp.tile([128, 128], BF, tag="xT")
                    nc.vector.tensor_copy(out=aggT[:DH, :], in_=tr[:DH, :])
                    o5 = ps_h.tile([128, DOUT], F32, space="PSUM", tag="h")
                    nc.tensor.matmul(out=o5[:], lhsT=aggT[:DH, :],
                                     rhs=W_t[5][:], start=True, stop=True)
                    nc.vector.tensor_copy(
                        out=out_sb[:, t * DOUT:(t + 1) * DOUT], in_=o5[:])

            for l in range(6):
                xsrc = xa if l % 2 == 1 else xb
                xdst = xb if l % 2 == 1 else xa
                # ---- z phase (per half, then ship its AllGather)
                for h in range(NHALF):
                    for t in range(h * TPH, (h + 1) * TPH):
                        if l == 0:
                            xT = x0p.tile([128, 128], BF, tag="x0")
                            nc.sync.dma_start(  # pre-transposed bf16 input
                                out=xT[:],
                                in_=xs_d[:, t * 128:(t + 1) * 128])
                            h_ps = ps_h.tile([128, DH], F32, space="PSUM",
                                             tag="h")
                            nc.tensor.matmul(out=h_ps[:], lhsT=xT[:],
                                             rhs=W_t[0][:], start=True,
                                             stop=True)
                            nc.vector.tensor_tensor(
                                out=hs_buf[:, t * DH:(t + 1) * DH],
                                in0=h_ps[:], in1=dinv_col(t), op=AL.mult)
                        elif l <= 4:
                            xt_ap = xsrc[:, t * DH:(t + 1) * DH]
                            tr = ps_tr.tile([128, 128], BF, space="PSUM",
                                            tag="tr")
                            nc.tensor.transpose(out=tr[:DH, :], in_=xt_ap,
                                                identity=identb[:])
                            xT = xtp.tile([128, 128], BF, tag="xT")
                            nc.vector.tensor_copy(out=xT[:DH, :],
                                                  in_=tr[:DH, :])
                            h_ps = ps_h.tile([128, DH], F32, space="PSUM",
                                             tag="h")
                            nc.tensor.matmul(out=h_ps[:], lhsT=xT[:DH, :],
                                             rhs=W_t[l][:], start=True,
                                             stop=True)
                            nc.vector.tensor_tensor(
                                out=hs_buf[:, t * DH:(t + 1) * DH],
                                in0=h_ps[:], in1=dinv_col(t), op=AL.mult)
                        else:
                            nc.vector.tensor_tensor(
                                out=hs_buf[:, t * DH:(t + 1) * DH],
                                in0=xsrc[:, t * DH:(t + 1) * DH],
                                in1=dinv_col(t), op=AL.mult)
                    # ship half h: pair row (t_local*64+k) half par <-
                    # partition (par*64+k) of tile t
                    for par in range(2):
                        nc.sync.dma_start(
                            out=ag_d[h][:].rearrange(
                                "(t k) (pi d) -> pi k t d", k=64, d=DH)[par],
                            in_=hs_buf[par * 64:(par + 1) * 64,
                                       h * TPH * DH:(h + 1) * TPH * DH]
                            .rearrange("k (t d) -> k t d", d=DH))
                    if ABLATE != "nocoll":
                        nc.gpsimd.collective_compute(
                            "AllGather", AL.bypass,
                            replica_groups=[list(range(R))],
                            ins=[ag_d[h][:]], outs=[tab_d[h][:]])

                # ---- gather + accumulate: all window-0 waves first so the
                # Pool gather stream never stalls on AG1 while AG0 work is
                # available; SBUF accs[] carries partials between waves.
                for h in range(NHALF):
                    for sp in meta["stripes"]:
                        win = sp["wins"][h]
                        gts = []
                        for (cc0, ncols) in win["chunks"]:
                            gt = gbuf.tile([128, CHUNKCOLS, 2 * DH], BF,
                                           tag="g")
                            ioff = cc0 * 128
                            ni = ncols * 128
                            if ABLATE != "nogather":
                                nc.gpsimd.dma_gather(
                                    out_ap=gt[:, 0:ncols, :],
                                    in_ap=tab_d[h][:],
                                    idxs_ap=idx_t[:16,
                                                  ioff // 16:(ioff + ni) // 16],
                                    num_idxs=ni, num_idxs_reg=ni,
                                    elem_size=2 * DH,
                                    single_packet=(ncols <= 8),
                                    queue_num=next_q())
                            gts.append((cc0, ncols, gt))

                        def seg_chunks(c0, n):
                            out = []
                            for (cc0, ncols, gt) in gts:
                                lo = max(c0, cc0)
                                hi = min(c0 + n, cc0 + ncols)
                                if lo < hi:
                                    out.append((gt, lo - cc0, hi - lo))
                            return out

                        for t in sp["tiles"]:
                            segs = win["segs"][t]
                            nk = sum(s[1] for s in segs)
                            if nk == 0:
                                if h == 0:
                                    nc.vector.tensor_copy(
                                        out=accs[:, t * DH:(t + 1) * DH],
                                        in_=hs_buf[:, t * DH:(t + 1) * DH])
                                else:
                                    epilogue(l, t, None, xdst)
                                continue
                            acc = ps_acc.tile([128, DH], F32, space="PSUM",
                                              tag="acc")
                            ki = 0
                            for (c0, n, par) in segs:
                                off = par * DH
                                for (gt, lc, ln) in seg_chunks(c0, n):
                                    for k in range(ln):
                                        nc.tensor.matmul(
                                            out=acc[:], lhsT=identb[:],
                                            rhs=gt[:, lc + k, off:off + DH],
                                            start=(ki == 0),
                                            stop=(ki == nk - 1))
                                        ki += 1
                            if h == 0:
                                # accs = acc + hs  (self-loop term folded in)
                                nc.vector.tensor_tensor(
                                    out=accs[:, t * DH:(t + 1) * DH],
                                    in0=acc[:],
                                    in1=hs_buf[:, t * DH:(t + 1) * DH],
                                    op=AL.add)
                            else:
                                epilogue(l, t, acc, xdst)

            nc.sync.dma_start(
                out=out_d[:].rearrange("(t p) d -> p t d", p=128),
                in_=out_sb[:].rearrange("p (t d) -> p t d", d=DOUT))
    nc.compile()
    return nc


# ------------------------------------------------------------------ runner
def kernel(**inputs):
    from concourse.bass_utils import run_bass_kernel_spmd

    edge_index = np.asarray(inputs["edge_index"])
    x = np.asarray(inputs["x"], dtype=np.float32)
    Ws = [np.asarray(inputs[f"W{i}"], dtype=np.float32) for i in range(6)]
    bs = [np.asarray(inputs[f"b{i}"], dtype=np.float32) for i in range(6)]
    with_bias = any(float(np.abs(b).max()) > 0 for b in bs[:5])

    ck = ("prog", edge_index.shape[1], with_bias,
          int(edge_index[0, :8].sum()), int(edge_index[1, :8].sum()))
    if ck not in _cache:
        meta = _build_graph(edge_index)
        nc = _build_program(meta, with_bias)
        _cache[ck] = (meta, nc)
    meta, nc = _cache[ck]

    import ml_dtypes

    xs = np.zeros((R * SHARD, DIN), np.float32)
    xs[meta["slot_of_node"]] = x
    xs = np.ascontiguousarray(
        xs.reshape(R, SHARD, DIN).transpose(0, 2, 1)
    ).astype(ml_dtypes.bfloat16)

    maps = []
    for r in range(R):
        m = {
            "xs": xs[r],
            "gidx": meta["idx_tiles"][r],
            "dinv": meta["dinv_rt"][r],
        }
        for i in range(6):
            m[f"W{i}"] = Ws[i]
        if with_bias:
            m["bb"] = np.repeat(np.stack(bs[:5])[:, None, :], 128, axis=1
                                ).reshape(5 * 128, DH).astype(np.float32)
        maps.append(m)

    global _last_maps
    _last_maps = maps
    res = run_bass_kernel_spmd(nc, maps, core_ids=list(range(R)))
    out_full = np.zeros((R * SHARD, DOUT), np.float32)
    for r in range(R):
        out_full[r * SHARD:r * SHARD + SHARD] = res.results[r]["out"]
    out = out_full[meta["slot_of_node"]]
    if float(np.abs(bs[5]).max()) > 0:
        out = out + bs[5][None, :]
    return out.astype(np.float32)
